# revision 1
# baseline (speedup 1.0000x reference)
# GGNN encoder kernel for Trainium2 (Bass/Tile), data-parallel over the
# batch dimension: 8 graphs -> 8 NeuronCores, one graph per core.
#
# Per-core computation (one graph):
#   type_e  = type_table[node_types]                       # [N, TD]
#   tok_e   = word_emb[node_token_ids]                     # [T, D]   (SWDGE dma_gather)
#   text_e  = segment_mean(tok_e, token_seg_ids)           # [N, D]   (PE matmul w/ pooling matrix)
#   h       = concat(type_e, text_e) @ fusion_w + b        # [N, D]
#   4 x GGNN layer:
#     m    = h @ Wl                                        # [N, D]
#     agg  = A @ m          (A dense adjacency, built host-side from edge list)
#     GRU(h, agg)
#   out     = mask * h
#
# Layout strategy: h, agg, gates are kept feature-major ("T" layout,
# [feat partitions, node free-dim]) so that the feature-contracting GRU
# matmuls can run directly; m is node-major for the node-contracting
# scatter matmul. Matmuls run as float32r (full fp32 storage, single-pass
# PE mode) for 4x throughput over plain fp32.

import functools

import numpy as np

import concourse.bass as bass
import concourse.mybir as mybir
import concourse.tile as tile
from concourse import bacc, bass_utils
from concourse.masks import make_identity

# Problem shapes (hardcoded: kernel must be self-contained).
B, N, T, D, TD, L = 8, 512, 2048, 768, 128, 4
V, TYPES = 30522, 64
MAX_NODE_LEN = 512
K3 = 3 * D            # 2304 stacked GRU gate rows
F = TD + D            # 896 fused embedding dim
P = 128               # partitions
NCH = N // P          # 4 node chunks
TCH = T // P          # 16 token chunks
DCH = D // P          # 6 feature chunks
FCH = F // P          # 7 fused-dim chunks
GCH = 3 * DCH         # 18 gate row chunks
BLK = N // TCH        # 32 nodes per token chunk (block-pooling case)
NF = 512              # free-dim tile (nodes)
GS = 4                # token gather splits
GT = T // GS          # tokens per gather split (512)
GC = GT // P          # 128-chunks per gather split (4)

f32 = mybir.dt.float32
f32r = mybir.dt.float32r
i32 = mybir.dt.int32
i16 = mybir.dt.int16

Sigmoid = mybir.ActivationFunctionType.Sigmoid
Tanh = mybir.ActivationFunctionType.Tanh
Ident = mybir.ActivationFunctionType.Identity


def build_nc(pool_wide: bool) -> bass.Bass:
    nc = bacc.Bacc(num_swdge_queues=2, dynamic_dma_scratch_size=32768)

    # All host-side tensors are pre-laid-out partition-major so every DMA is
    # contiguous per partition.
    tok_idx = nc.dram_tensor("tok_idx", [P, GS * (GT // 16)], i16,
                             kind="ExternalInput")  # [128, 4*32] wrapped idxs
    typ_oh = nc.dram_tensor("typ_oh", [TYPES, N], f32r, kind="ExternalInput")
    word_emb = nc.dram_tensor("word_emb", [V, D], f32r, kind="ExternalInput")
    type_table = nc.dram_tensor("type_table", [TYPES, TD], f32r, kind="ExternalInput")
    pool_w = N if pool_wide else BLK
    poolm = nc.dram_tensor("poolm", [P, TCH, pool_w], f32r, kind="ExternalInput")
    at_w = nc.dram_tensor("at_w", [P, NCH, N], f32r, kind="ExternalInput")
    fusion_w = nc.dram_tensor("fusion_w", [F, D], f32r, kind="ExternalInput")
    fusion_b = nc.dram_tensor("fusion_b", [P, DCH], f32, kind="ExternalInput")
    wl = nc.dram_tensor("wl", [L, DCH, P, D], f32r, kind="ExternalInput")
    wih = nc.dram_tensor("wih", [P, DCH, K3], f32r, kind="ExternalInput")
    whh_st = nc.dram_tensor("whh_st", [GCH, P, DCH, P], f32r, kind="ExternalInput")
    bsum = nc.dram_tensor("bsum", [P, GCH], f32, kind="ExternalInput")
    bihn = nc.dram_tensor("bihn", [P, DCH], f32, kind="ExternalInput")
    bhhn = nc.dram_tensor("bhhn", [P, DCH], f32, kind="ExternalInput")
    maskc = nc.dram_tensor("maskc", [P, NCH], f32, kind="ExternalInput")
    out = nc.dram_tensor("out", [N, D], f32, kind="ExternalOutput")

    with tile.TileContext(nc) as tc:
        with (
            tc.tile_pool(name="consts", bufs=1) as consts,
            tc.tile_pool(name="wbig", bufs=1) as wbig,
            tc.tile_pool(name="t768", bufs=7) as t768,
            tc.tile_pool(name="c512", bufs=7) as c512,
            tc.tile_pool(name="hpool", bufs=12) as hpool,
            tc.tile_pool(name="gpool", bufs=5) as gpool,
            tc.tile_pool(name="wst", bufs=3) as wst,
            tc.tile_pool(name="wlc", bufs=7) as wlc,
            tc.tile_pool(name="tokg", bufs=2) as tokg,
            tc.tile_pool(name="psA", bufs=7, space="PSUM") as psA,
        ):
            # ---- token gather first: it gates the whole front of the kernel
            tok_idx_sb = consts.tile([P, T // 16], i16)
            nc.sync.dma_start(out=tok_idx_sb[:], in_=tok_idx[:])
            pool_sb = consts.tile([P, TCH, pool_w], f32r)
            nc.sync.dma_start(out=pool_sb[:], in_=poolm[:])

            # type embeddings via one-hot matmul: two tiny DMAs + one PE op,
            # nothing queues behind the big token gathers
            tt_sb = consts.tile([TYPES, TD], f32r)
            nc.sync.dma_start(out=tt_sb[:], in_=type_table[:])
            oh_sb = consts.tile([TYPES, N], f32r)
            nc.sync.dma_start(out=oh_sb[:], in_=typ_oh[:])

            gath = []
            gath_insts = []
            for s in range(GS):
                tg = tokg.tile([P, GC, D], f32r, tag="tokg", name=f"tokg{s}")
                gi_ = nc.gpsimd.dma_gather(
                    tg[:],
                    word_emb[:],
                    tok_idx_sb[:, s * (GT // 16) : (s + 1) * (GT // 16)],
                    GT,
                    GT,
                    D,
                    queue_num=s % 2,
                )
                gath.append(tg)
                gath_insts.append(gi_)

            def after_gathers(dma_inst):
                return dma_inst

            # ---- remaining constants / small inputs ----
            identity = consts.tile([P, P], f32)
            make_identity(nc, identity[:])
            bsum_sb = consts.tile([P, GCH], f32)
            nc.sync.dma_start(out=bsum_sb[:], in_=bsum[:])
            bihn_sb = consts.tile([P, DCH], f32)
            nc.sync.dma_start(out=bihn_sb[:], in_=bihn[:])
            bhhn_sb = consts.tile([P, DCH], f32)
            nc.sync.dma_start(out=bhhn_sb[:], in_=bhhn[:])
            fb_sb = consts.tile([P, DCH], f32)
            nc.sync.dma_start(out=fb_sb[:], in_=fusion_b[:])
            mask_sb = consts.tile([P, NCH], f32)
            nc.sync.dma_start(out=mask_sb[:], in_=maskc[:])

            # ---- fused embedding (feature-major [f, n]) ----
            fusedT = [
                c512.tile([P, NF], f32r, tag="c512", name=f"fusedT{k}")
                for k in range(FCH)
            ]

            # weight loads, emitted in the order the compute will need them
            # (the DMA engines drain roughly in emission order)
            fw = []
            for k in range(FCH):
                fwk = t768.tile([P, D], f32r, tag="t768", name=f"fw{k}")
                after_gathers(nc.scalar.dma_start(
                    out=fwk[:], in_=fusion_w[k * P : (k + 1) * P, :]
                ))
                fw.append(fwk)
            wlk = []
            for k in range(DCH):
                wk = wlc.tile([P, D], f32r, tag="wlc", name=f"wl0_{k}")
                after_gathers(nc.scalar.dma_start(out=wk[:], in_=wl[0, k]))
                wlk.append(wk)
            at_sb = wbig.tile([P, NCH, N], f32r)
            after_gathers(nc.scalar.dma_start(out=at_sb[:], in_=at_w[:]))
            wih_sb = wbig.tile([P, DCH, K3], f32r)

            # type_eT = type_table.T @ onehot  (one matmul, K=64)
            ptyp = psA.tile([P, NF], f32, tag="psA")
            nc.tensor.matmul(
                out=ptyp[:], lhsT=tt_sb[:], rhs=oh_sb[:], start=True, stop=True
            )
            nc.vector.tensor_copy(out=fusedT[0][:], in_=ptyp[:])

            # token pooling: PE matmul pools 128 tokens -> 32 nodes and
            # transposes to feature-major in one pass
            for s in range(GS):
                tg = gath[s]
                for c2 in range(GC):
                    c = s * GC + c2
                    if pool_wide:
                        for f in range(DCH):
                            pc = psA.tile([P, NF], f32, tag="psA")
                            nc.tensor.matmul(
                                out=pc[:],
                                lhsT=tg[:, c2, f * P : (f + 1) * P],
                                rhs=pool_sb[:, c, :],
                                start=True,
                                stop=True,
                            )
                            if c == 0:
                                nc.vector.tensor_copy(out=fusedT[1 + f][:], in_=pc[:])
                            else:
                                nc.vector.tensor_add(
                                    out=fusedT[1 + f][:],
                                    in0=fusedT[1 + f][:],
                                    in1=pc[:],
                                )
                    else:
                        pc = psA.tile([P, DCH * BLK], f32, tag="psA")
                        for f in range(DCH):
                            nc.tensor.matmul(
                                out=pc[:, f * BLK : (f + 1) * BLK],
                                lhsT=tg[:, c2, f * P : (f + 1) * P],
                                rhs=pool_sb[:, c, :],
                                start=True,
                                stop=True,
                            )
                        for f in range(DCH):
                            nc.vector.tensor_copy(
                                out=fusedT[1 + f][:, c * BLK : (c + 1) * BLK],
                                in_=pc[:, f * BLK : (f + 1) * BLK],
                            )

            # ---- fusion matmul: hT[j] = (fusion_w.T @ fusedT)[j] + b ----
            hT = []
            for j in range(DCH):
                pf = psA.tile([P, NF], f32, tag="psA")
                for k in range(FCH):
                    nc.tensor.matmul(
                        out=pf[:],
                        lhsT=fw[k][:, j * P : (j + 1) * P],
                        rhs=fusedT[k][:],
                        start=(k == 0),
                        stop=(k == FCH - 1),
                    )
                hj = hpool.tile([P, NF], f32r, tag="hpool")
                nc.scalar.activation(
                    out=hj[:], in_=pf[:], func=Ident, bias=fb_sb[:, j : j + 1]
                )
                hT.append(hj)
                after_gathers(nc.scalar.dma_start(out=wih_sb[:, j, :], in_=wih[:, j, :]))

            # ---- GGNN layers ----
            for l in range(L):
                # m = h @ Wl   (node-major out, [node 128, 768] per chunk)
                if l > 0:
                    wlk = []
                    for k in range(DCH):
                        wk = wlc.tile([P, D], f32r, tag="wlc", name=f"wl{l}_{k}")
                        nc.scalar.dma_start(out=wk[:], in_=wl[l, k])
                        wlk.append(wk)
                m_sb = []
                for i in range(NCH):
                    pma = psA.tile([P, NF], f32, tag="psA")
                    pmb = psA.tile([P, D - NF], f32, tag="psA")
                    for k in range(DCH):
                        nc.tensor.matmul(
                            out=pma[:],
                            lhsT=hT[k][:, i * P : (i + 1) * P],
                            rhs=wlk[k][:, :NF],
                            start=(k == 0),
                            stop=(k == DCH - 1),
                        )
                        nc.tensor.matmul(
                            out=pmb[:],
                            lhsT=hT[k][:, i * P : (i + 1) * P],
                            rhs=wlk[k][:, NF:D],
                            start=(k == 0),
                            stop=(k == DCH - 1),
                        )
                    mi = t768.tile([P, D], f32r, tag="t768", name=f"m{l}_{i}")
                    nc.vector.tensor_copy(out=mi[:, :NF], in_=pma[:])
                    nc.vector.tensor_copy(out=mi[:, NF:D], in_=pmb[:])
                    m_sb.append(mi)

                # aggT = m.T @ A.T  (feature-major [feat 128, nodes 512])
                aggT = []
                for j in range(DCH):
                    pa = psA.tile([P, NF], f32, tag="psA")
                    for k in range(NCH):
                        nc.tensor.matmul(
                            out=pa[:],
                            lhsT=m_sb[k][:, j * P : (j + 1) * P],
                            rhs=at_sb[:, k, :],
                            start=(k == 0),
                            stop=(k == NCH - 1),
                        )
                    aj = c512.tile([P, NF], f32r, tag="c512", name=f"agg{l}_{j}")
                    nc.vector.tensor_copy(out=aj[:], in_=pa[:])
                    aggT.append(aj)

                # GRU gates, 128 gate rows at a time
                hnew = []
                for i in range(DCH):
                    # streamed Whh chunks for the three gates at row-chunk i
                    wch = []
                    for g in range(3):
                        w = wst.tile([P, DCH, P], f32r, tag="wst",
                                     name=f"wch{l}_{i}_{g}")
                        wdma = nc.sync.dma_start(out=w[:], in_=whh_st[g * DCH + i])
                        if l == 0 and i == 0:
                            after_gathers(wdma)
                        wch.append(w)

                    # r and z: psum accumulates gi + gh, ACT adds bias+sigmoid
                    rz = []
                    for g in range(2):
                        pg = psA.tile([P, NF], f32, tag="psA")
                        col = g * D + i * P
                        # gh first: it only needs h + the small whh stream,
                        # so it runs while wih/aggT are still in flight
                        for k in range(DCH):
                            nc.tensor.matmul(
                                out=pg[:],
                                lhsT=wch[g][:, k, :],
                                rhs=hT[k][:],
                                start=(k == 0),
                                stop=False,
                            )
                        for k in range(DCH):
                            nc.tensor.matmul(
                                out=pg[:],
                                lhsT=wih_sb[:, k, col : col + P],
                                rhs=aggT[k][:],
                                start=False,
                                stop=(k == DCH - 1),
                            )
                        gs = gpool.tile([P, NF], f32, tag="gpool",
                                        name=f"g{l}_{i}_{g}")
                        nc.scalar.activation(
                            out=gs[:],
                            in_=pg[:],
                            func=Sigmoid,
                            bias=bsum_sb[:, g * DCH + i : g * DCH + i + 1],
                        )
                        rz.append(gs)
                    r_sb, z_sb = rz

                    # n gate: keep gi and gh separate
                    col = 2 * D + i * P
                    pghn = psA.tile([P, NF], f32, tag="psA")
                    for k in range(DCH):
                        nc.tensor.matmul(
                            out=pghn[:],
                            lhsT=wch[2][:, k, :],
                            rhs=hT[k][:],
                            start=(k == 0),
                            stop=(k == DCH - 1),
                        )
                    pgin = psA.tile([P, NF], f32, tag="psA")
                    for k in range(DCH):
                        nc.tensor.matmul(
                            out=pgin[:],
                            lhsT=wih_sb[:, k, col : col + P],
                            rhs=aggT[k][:],
                            start=(k == 0),
                            stop=(k == DCH - 1),
                        )
                    hb = gpool.tile([P, NF], f32, tag="gpool")
                    nc.scalar.activation(
                        out=hb[:], in_=pghn[:], func=Ident,
                        bias=bhhn_sb[:, i : i + 1],
                    )
                    rn = gpool.tile([P, NF], f32, tag="gpool")
                    nc.vector.tensor_mul(out=rn[:], in0=r_sb[:], in1=hb[:])
                    tn = gpool.tile([P, NF], f32, tag="gpool")
                    nc.vector.tensor_add(out=tn[:], in0=pgin[:], in1=rn[:])
                    nn_ = gpool.tile([P, NF], f32, tag="gpool")
                    nc.scalar.activation(
                        out=nn_[:], in_=tn[:], func=Tanh,
                        bias=bihn_sb[:, i : i + 1],
                    )
                    # h' = n + z * (h - n)
                    s_ = gpool.tile([P, NF], f32, tag="gpool")
                    nc.vector.tensor_sub(out=s_[:], in0=hT[i][:], in1=nn_[:])
                    sz = gpool.tile([P, NF], f32, tag="gpool")
                    nc.vector.tensor_mul(out=sz[:], in0=z_sb[:], in1=s_[:])
                    hj = hpool.tile([P, NF], f32r, tag="hpool",
                                    name=f"h{l}_{i}")
                    nc.vector.tensor_add(out=hj[:], in0=nn_[:], in1=sz[:])
                    hnew.append(hj)
                hT = hnew

            # ---- transpose back to node-major, mask, write out ----
            for i in range(NCH):
                poa = psA.tile([P, NF], f32, tag="psA")
                pob = psA.tile([P, D - NF], f32, tag="psA")
                for j in range(DCH):
                    dst = poa[:, j * P : (j + 1) * P] if j < 4 else \
                        pob[:, (j - 4) * P : (j - 3) * P]
                    nc.tensor.transpose(
                        out=dst,
                        in_=hT[j][:, i * P : (i + 1) * P].bitcast(f32),
                        identity=identity[:],
                    )
                ob = t768.tile([P, D], f32, tag="t768")
                nc.vector.tensor_scalar_mul(
                    out=ob[:, :NF], in0=poa[:], scalar1=mask_sb[:, i : i + 1]
                )
                nc.vector.tensor_scalar_mul(
                    out=ob[:, NF:D], in0=pob[:], scalar1=mask_sb[:, i : i + 1]
                )
                nc.sync.dma_start(out=out[i * P : (i + 1) * P, :], in_=ob[:])

    nc.compile()
    return nc


@functools.lru_cache(maxsize=2)
def _get_nc(pool_wide: bool) -> bass.Bass:
    return build_nc(pool_wide)


def _prep_shared(inputs):
    """Weight tensors identical across graphs, pre-laid-out partition-major."""
    fusion_w = np.ascontiguousarray(np.asarray(inputs["fusion_w"], np.float32))
    fusion_b = np.ascontiguousarray(
        np.asarray(inputs["fusion_b"], np.float32).reshape(DCH, P).T
    )
    wl = np.ascontiguousarray(
        np.asarray(inputs["ggnn_w"], np.float32).reshape(L, DCH, P, D)
    )
    wih_w = np.asarray(inputs["gru_w_ih"], np.float32)   # [K3, D]
    whh_w = np.asarray(inputs["gru_w_hh"], np.float32)
    bih = np.asarray(inputs["gru_b_ih"], np.float32)
    bhh = np.asarray(inputs["gru_b_hh"], np.float32)
    # wih: [P, DCH, K3]  (partition p, feat chunk k -> gate rows)
    wihT = wih_w.T                                       # [D, K3]
    wih = np.ascontiguousarray(wihT.reshape(DCH, P, K3).transpose(1, 0, 2))
    # whh chunks: [GCH, P, DCH, P]
    whhT = whh_w.T                                       # [D, K3]
    whh_st = np.ascontiguousarray(
        np.stack(
            [
                whhT[:, j * P : (j + 1) * P].reshape(DCH, P, P).transpose(1, 0, 2)
                for j in range(GCH)
            ]
        )
    )
    bsum = np.ascontiguousarray((bih + bhh).reshape(GCH, P).T)
    bihn = np.ascontiguousarray(bih[2 * D :].reshape(DCH, P).T)
    bhhn = np.ascontiguousarray(bhh[2 * D :].reshape(DCH, P).T)
    word_emb = np.ascontiguousarray(np.asarray(inputs["word_emb"], np.float32))
    type_table = np.ascontiguousarray(np.asarray(inputs["type_table"], np.float32))
    return dict(
        word_emb=word_emb, type_table=type_table, fusion_w=fusion_w,
        fusion_b=fusion_b, wl=wl, wih=wih, whh_st=whh_st, bsum=bsum,
        bihn=bihn, bhhn=bhhn,
    )


def _graph_blockable(inputs, b):
    seg = np.asarray(inputs["token_seg_ids"][b], np.int64)
    tcol = np.arange(T) // P
    return bool(np.all((seg >= tcol * BLK) & (seg < (tcol + 1) * BLK)))


def _prep_graph(inputs, b, pool_wide):
    tok = np.asarray(inputs["node_token_ids"][b], np.int64)
    typ = np.asarray(inputs["node_types"][b], np.int32)
    seg = np.asarray(inputs["token_seg_ids"][b], np.int64)
    lens = np.asarray(inputs["node_token_lens"][b], np.float64)
    glen = int(np.asarray(inputs["graph_node_lens"][b]))
    esrc = np.asarray(inputs["edge_src"][b], np.int64)
    edst = np.asarray(inputs["edge_dst"][b], np.int64)
    ew = np.asarray(inputs["edge_weight"][b], np.float32)

    # token idxs for dma_gather: GS splits of GT idxs, each wrapped into
    # 16 partitions ([p, s] = idx[s*16+p]) and replicated to 128 partitions
    tok16 = tok.astype(np.int16)
    cols = []
    for s in range(GS):
        w16 = tok16[s * GT : (s + 1) * GT].reshape(GT // 16, 16).T  # [16, GT/16]
        cols.append(np.tile(w16, (8, 1)))                           # [128, GT/16]
    tok_idx = np.ascontiguousarray(np.concatenate(cols, axis=1))    # [128, GS*32]

    typ_oh = np.zeros((TYPES, N), np.float32)
    typ_oh[typ, np.arange(N)] = 1.0

    # dense transposed adjacency: AT[src, dst], laid out [P, NCH, N]
    at = np.zeros((N, N), np.float32)
    np.add.at(at, (esrc, edst), ew)
    at = np.ascontiguousarray(at.reshape(NCH, P, N).transpose(1, 0, 2))

    # pooling matrix (1/len weights), [P, TCH, BLK or N]
    winv = np.zeros(N, np.float64)
    nzmask = lens != 0
    winv[nzmask] = 1.0 / lens[nzmask]
    tcol = np.arange(T) // P  # token chunk of each token
    if pool_wide:
        poolm = np.zeros((TCH, P, N), np.float32)
        poolm[tcol, np.arange(T) % P, seg] = winv[seg]
    else:
        poolm = np.zeros((TCH, P, BLK), np.float32)
        poolm[tcol, np.arange(T) % P, seg - tcol * BLK] = winv[seg]
    poolm = np.ascontiguousarray(poolm.transpose(1, 0, 2))

    keep = min(glen, MAX_NODE_LEN)
    mask = np.ascontiguousarray(
        (np.arange(N) < keep).astype(np.float32).reshape(NCH, P).T
    )
    return dict(tok_idx=tok_idx, typ_oh=typ_oh, at_w=at, poolm=poolm,
                maskc=mask)


def kernel(**inputs) -> np.ndarray:
    shared = _prep_shared(inputs)
    pool_wide = not all(_graph_blockable(inputs, b) for b in range(B))
    per_graph = [_prep_graph(inputs, b, pool_wide) for b in range(B)]
    nc = _get_nc(pool_wide)
    in_maps = [{**shared, **per_graph[b]} for b in range(B)]
    res = bass_utils.run_bass_kernel_spmd(nc, in_maps, core_ids=list(range(B)))
    global _last_exec_ns
    _last_exec_ns = res.exec_time_ns
    out = np.stack([r["out"] for r in res.results]).astype(np.float32)
    return out


_last_exec_ns = None



# revision 2
# speedup vs baseline: 1.0432x; 1.0432x over previous
# GGNN encoder kernel for Trainium2 (Bass/Tile), data-parallel over the
# batch dimension: 8 graphs -> 8 NeuronCores, one graph per core.
#
# Numerics strategy (validated against the reference in numpy):
#   - all tensors stored fp16 in SBUF, matmuls accumulate in f32 PSUM
#   - the GRU r/z gate matmuls and the gh_n matmul run in fp8e4 (e4m3)
#     with DoubleRow perf mode (2 contraction rows per pass, 2x PE rate);
#     these paths only perturb the output at second order (sigmoid gates
#     multiply small residuals; ghn is damped by r) - measured final
#     rel err ~6.5e-3 vs the 2e-2 gate.
#   - the additive n-path (m, agg, gi_n, fusion, pooling) stays fp16.
#   fp8 scales: activations x16, weights x256, descale 2^-12 folded into
#   the ACT scale of the consuming sigmoid/ident op.
#
# Layout: h, agg feature-major fp16 [128, DCH, 512] with fp8 shadows for
# the DoubleRow matmuls; m node-major fp16 [128, NCH, 768]. All weights
# loaded once (fp8/fp16), never streamed per layer. Output written
# feature-major and transposed on the host.
#
# Engine assignment: PE matmuls; ACT sigmoid/tanh/descale + agg8 cast;
# DVE the GRU elementwise chain (fp16 2x mode); Pool(gpsimd) the
# psum->sbuf evacuations and fp8 casts; SP the weight/output DMAs.

import functools

import numpy as np

import concourse.bass as bass
import concourse.mybir as mybir
import concourse.tile as tile
from concourse import bacc, bass_utils

# Problem shapes (hardcoded: kernel must be self-contained).
B, N, T, D, TD, L = 8, 512, 2048, 768, 128, 4
V, TYPES = 30522, 64
MAX_NODE_LEN = 512
K3 = 3 * D            # 2304 stacked GRU gate rows
F = TD + D            # 896 fused embedding dim
P = 128               # partitions
NCH = N // P          # 4 node chunks
TCH = T // P          # 16 token chunks
DCH = D // P          # 6 feature chunks
FCH = F // P          # 7 fused-dim chunks
BLK = N // TCH        # 32 nodes per token chunk (block-pooling case)
NF = 512              # free-dim tile (nodes)
GS = 4                # token gather splits
GT = T // GS          # tokens per gather split (512)
GC = GT // P          # 128-chunks per gather split (4)

S_A = 16.0            # fp8 activation scale (2^4)
S_W = 256.0           # fp8 weight scale (2^8)
DESC = 1.0 / (S_A * S_W)

f32 = mybir.dt.float32
f16 = mybir.dt.float16
f8 = mybir.dt.float8e4
i16 = mybir.dt.int16

DR = mybir.MatmulPerfMode.DoubleRow
Sigmoid = mybir.ActivationFunctionType.Sigmoid
Tanh = mybir.ActivationFunctionType.Tanh
Ident = mybir.ActivationFunctionType.Identity
Copy = mybir.ActivationFunctionType.Copy
ADD = mybir.AluOpType.add
MULT = mybir.AluOpType.mult


def build_nc(pool_wide: bool, zero_nb: bool = True) -> bass.Bass:
    nc = bacc.Bacc(num_swdge_queues=2, dynamic_dma_scratch_size=32768)

    tok_idx = nc.dram_tensor("tok_idx", [P, GS * (GT // 16)], i16,
                             kind="ExternalInput")
    typ_oh = nc.dram_tensor("typ_oh", [TYPES, N], f16, kind="ExternalInput")
    word_emb = nc.dram_tensor("word_emb", [V, D], f16, kind="ExternalInput")
    type_table = nc.dram_tensor("type_table", [TYPES, TD], f16,
                                kind="ExternalInput")
    pool_w = N if pool_wide else BLK
    poolm = nc.dram_tensor("poolm", [P, TCH, pool_w], f16, kind="ExternalInput")
    at_w = nc.dram_tensor("at_w", [P, NCH, N], f16, kind="ExternalInput")
    fusion_w = nc.dram_tensor("fusion_w", [F, D], f16, kind="ExternalInput")
    fusion_b = nc.dram_tensor("fusion_b", [P, DCH], f32, kind="ExternalInput")
    wl = nc.dram_tensor("wl", [L, P, DCH, D], f16, kind="ExternalInput")
    wihn = nc.dram_tensor("wihn", [P, DCH, D], f16, kind="ExternalInput")
    wih8d = nc.dram_tensor("wih8", [P, DCH, 2 * D], f8, kind="ExternalInput")
    whh8d = nc.dram_tensor("whh8", [P, DCH, K3], f8, kind="ExternalInput")
    bsum = nc.dram_tensor("bsum", [P, 2 * DCH], f32, kind="ExternalInput")
    bihn = nc.dram_tensor("bihn", [P, DCH], f32, kind="ExternalInput")
    bhhn = nc.dram_tensor("bhhn", [P, DCH], f32, kind="ExternalInput")
    maskb = nc.dram_tensor("maskb", [P, N], f16, kind="ExternalInput")
    out = nc.dram_tensor("out", [D, N], f32, kind="ExternalOutput")

    with tile.TileContext(nc) as tc:
        with (
            tc.tile_pool(name="consts", bufs=1) as consts,
            tc.tile_pool(name="wpool", bufs=1) as wpool,
            tc.tile_pool(name="fpool", bufs=1) as fpool,
            tc.tile_pool(name="hpool", bufs=2) as hpool,
            tc.tile_pool(name="apool", bufs=2) as apool,
            tc.tile_pool(name="mpool", bufs=2) as mpool,
            tc.tile_pool(name="gpool", bufs=14) as gpool,
            tc.tile_pool(name="tokg", bufs=4) as tokg,
            tc.tile_pool(name="opool", bufs=2) as opool,
            tc.tile_pool(name="psA", bufs=5, space="PSUM") as psA,
        ):
            # gh psums (tag psG) get their own 3-slot ring so the early-gh
            # emission can pin slots across the agg phase without deadlocking
            # the main psum ring
            def gh_psum(name):
                return psA.tile([P, NF], f32, tag="psG", bufs=3, name=name)
            # ---- token gather first: it gates the whole front of the kernel
            tok_idx_sb = consts.tile([P, T // 16], i16)
            nc.sync.dma_start(out=tok_idx_sb[:], in_=tok_idx[:])
            pool_sb = consts.tile([P, TCH, pool_w], f16)
            nc.sync.dma_start(out=pool_sb[:], in_=poolm[:])

            tt_sb = consts.tile([TYPES, TD], f16)
            nc.sync.dma_start(out=tt_sb[:], in_=type_table[:])
            oh_sb = consts.tile([TYPES, N], f16)
            nc.sync.dma_start(out=oh_sb[:], in_=typ_oh[:])

            gath = []
            for s in range(GS):
                tg = tokg.tile([P, GC, D], f16, tag="tokg", name=f"tokg{s}")
                nc.gpsimd.dma_gather(
                    tg[:],
                    word_emb[:],
                    tok_idx_sb[:, s * (GT // 16): (s + 1) * (GT // 16)],
                    GT,
                    GT,
                    D,
                    queue_num=s % 2,
                )
                gath.append(tg)

            # ---- remaining constants / weights (after the gathers so the
            # gathers win the DMA engines first) ----
            bsum_sb = consts.tile([P, 2 * DCH], f32)
            nc.sync.dma_start(out=bsum_sb[:], in_=bsum[:])
            bihn_sb = consts.tile([P, DCH], f32)
            nc.sync.dma_start(out=bihn_sb[:], in_=bihn[:])
            bhhn_sb = consts.tile([P, DCH], f32)
            nc.sync.dma_start(out=bhhn_sb[:], in_=bhhn[:])
            fb_sb = consts.tile([P, DCH], f32)
            nc.sync.dma_start(out=fb_sb[:], in_=fusion_b[:])
            mask_sb = consts.tile([P, N], f16)
            nc.sync.dma_start(out=mask_sb[:], in_=maskb[:])

            fw = []
            for k in range(FCH):
                fwk = fpool.tile([P, D], f16, tag="fw", bufs=FCH,
                                 name=f"fw{k}")
                nc.sync.dma_start(out=fwk[:], in_=fusion_w[k * P: (k + 1) * P, :])
                fw.append(fwk)
            # weights layer 0 needs first, then the rest
            wl_sb = []
            w0 = wpool.tile([P, DCH, D], f16, tag="wl", bufs=L, name="wl0")
            nc.sync.dma_start(out=w0[:], in_=wl[0])
            wl_sb.append(w0)
            at_sb = wpool.tile([P, NCH, N], f16, tag="at")
            nc.sync.dma_start(out=at_sb[:], in_=at_w[:])
            whh8 = wpool.tile([P, DCH, K3], f8, tag="whh8")
            nc.sync.dma_start(out=whh8[:], in_=whh8d[:])
            wih8 = wpool.tile([P, DCH, 2 * D], f8, tag="wih8")
            nc.sync.dma_start(out=wih8[:], in_=wih8d[:])
            wihn_sb = wpool.tile([P, DCH, D], f16, tag="wihn")
            nc.sync.dma_start(out=wihn_sb[:], in_=wihn[:])
            for l in range(1, L):
                w = wpool.tile([P, DCH, D], f16, tag="wl", bufs=L,
                               name=f"wl{l}")
                nc.sync.dma_start(out=w[:], in_=wl[l])
                wl_sb.append(w)

            # ---- pooling: per-feature psums accumulate all token chunks;
            # one evac per feature chunk. fusedT[k] is [feat 128, nodes 512].
            fusedT = [fpool.tile([P, NF], f16, tag="fu", bufs=FCH,
                                 name=f"fusedT{k}") for k in range(FCH)]

            ptyp = psA.tile([P, NF], f32, tag="psA")
            nc.tensor.matmul(out=ptyp[:], lhsT=tt_sb[:], rhs=oh_sb[:],
                             start=True, stop=True)
            nc.vector.tensor_copy(out=fusedT[0][:], in_=ptyp[:])

            for f in range(DCH):
                pcf = psA.tile([P, NF], f32, tag="psG", bufs=3,
                               name=f"pool{f}")
                for s in range(GS):
                    for c2 in range(GC):
                        c = s * GC + c2
                        if pool_wide:
                            nc.tensor.matmul(
                                out=pcf[:],
                                lhsT=gath[s][:, c2, f * P: (f + 1) * P],
                                rhs=pool_sb[:, c, :],
                                start=(c == 0),
                                stop=(c == TCH - 1),
                            )
                        else:
                            nc.tensor.matmul(
                                out=pcf[:, c * BLK: (c + 1) * BLK],
                                lhsT=gath[s][:, c2, f * P: (f + 1) * P],
                                rhs=pool_sb[:, c, :],
                                start=True,
                                stop=True,
                            )
                nc.vector.tensor_copy(out=fusedT[1 + f][:], in_=pcf[:])

            # ---- fusion matmul: h[j] = (fusion_w.T @ fusedT)[j] + b ----
            h16 = hpool.tile([P, DCH, NF], f16, tag="h16", name="h_init")
            h8 = hpool.tile([P, DCH, NF], f8, tag="h8", name="h8_init")
            for j in range(DCH):
                pf = psA.tile([P, NF], f32, tag="psA")
                for k in range(FCH):
                    nc.tensor.matmul(
                        out=pf[:],
                        lhsT=fw[k][:, j * P: (j + 1) * P],
                        rhs=fusedT[k][:],
                        start=(k == 0),
                        stop=(k == FCH - 1),
                    )
                nc.vector.tensor_scalar(
                    out=h16[:, j, :], in0=pf[:],
                    scalar1=fb_sb[:, j: j + 1], scalar2=None, op0=ADD,
                )
                nc.gpsimd.tensor_scalar_mul(
                    out=h8[:, j, :], in0=h16[:, j, :], scalar1=S_A,
                )

            # ---- GGNN layers ----
            def emit_gh(i, h8t, lbl):
                """fp8 DoubleRow gh matmuls for gate chunk i; r/z psums stay
                open (gi accumulates into them later)."""
                przn = []
                for g in range(3):
                    pg = gh_psum(f"gh{lbl}_{g}_{i}")
                    col = g * D + i * P
                    for k in range(0, DCH, 2):
                        nc.tensor.matmul(
                            out=pg[:],
                            lhsT=whh8[:, k: k + 2, col: col + P],
                            rhs=h8t[:, k: k + 2, :],
                            start=(k == 0),
                            stop=(g == 2 and k + 2 == DCH),
                            perf_mode=DR,
                        )
                    przn.append(pg)
                return przn

            for l in range(L):
                wlk = wl_sb[l]
                last = l == L - 1

                # m = h @ Wl  (node-major out [node 128, 768] per chunk)
                m16 = mpool.tile([P, NCH, D], f16, tag="m16", name=f"m{l}")
                for i in range(NCH):
                    pma = psA.tile([P, NF], f32, tag="psA")
                    pmb = psA.tile([P, D - NF], f32, tag="psA")
                    for k in range(DCH):
                        nc.tensor.matmul(
                            out=pma[:],
                            lhsT=h16[:, k, i * P: (i + 1) * P],
                            rhs=wlk[:, k, :NF],
                            start=(k == 0),
                            stop=(k == DCH - 1),
                        )
                        nc.tensor.matmul(
                            out=pmb[:],
                            lhsT=h16[:, k, i * P: (i + 1) * P],
                            rhs=wlk[:, k, NF:D],
                            start=(k == 0),
                            stop=(k == DCH - 1),
                        )
                    nc.scalar.activation(out=m16[:, i, :NF], in_=pma[:],
                                         func=Copy)
                    nc.scalar.activation(out=m16[:, i, NF:D], in_=pmb[:],
                                         func=Copy)

                # gh for chunk 0: needs only h8, keeps the PE busy while the
                # m evacuations drain before agg can start
                gh_next = emit_gh(0, h8, l)

                # aggT = m.T @ A.T  (feature-major [feat 128, nodes 512])
                agg16 = apool.tile([P, DCH, NF], f16, tag="a16",
                                   name=f"agg{l}")
                agg8 = apool.tile([P, DCH, NF], f8, tag="a8",
                                  name=f"agg8_{l}")
                for j in range(DCH):
                    pa = psA.tile([P, NF], f32, tag="psA")
                    for k in range(NCH):
                        nc.tensor.matmul(
                            out=pa[:],
                            lhsT=m16[:, k, j * P: (j + 1) * P],
                            rhs=at_sb[:, k, :],
                            start=(k == 0),
                            stop=(k == NCH - 1),
                        )
                    nc.scalar.activation(out=agg8[:, j, :], in_=pa[:],
                                         func=Copy, scale=S_A)
                    nc.vector.tensor_copy(out=agg16[:, j, :], in_=pa[:])

                # GRU gates, 128 gate rows at a time
                h16n = hpool.tile([P, DCH, NF], f16, tag="h16",
                                  name=f"h{l + 1}")
                h8n = hpool.tile([P, DCH, NF], f8, tag="h8",
                                 name=f"h8_{l + 1}")
                for i in range(DCH):
                    pr, pz, pghn = gh_next
                    # gi_r, gi_z: fp8 DoubleRow accumulating into gh psums
                    for g, pg in ((0, pr), (1, pz)):
                        col = g * D + i * P
                        for k in range(0, DCH, 2):
                            nc.tensor.matmul(
                                out=pg[:],
                                lhsT=wih8[:, k: k + 2, col: col + P],
                                rhs=agg8[:, k: k + 2, :],
                                start=False,
                                stop=(k + 2 == DCH),
                                perf_mode=DR,
                            )
                    # gi_n: fp16 (precision-critical additive path)
                    pgn = psA.tile([P, NF], f32, tag="psA", name=f"pgin{i}")
                    for k in range(DCH):
                        nc.tensor.matmul(
                            out=pgn[:],
                            lhsT=wihn_sb[:, k, i * P: (i + 1) * P],
                            rhs=agg16[:, k, :],
                            start=(k == 0),
                            stop=(k == DCH - 1),
                        )
                    if i + 1 < DCH:
                        gh_next = emit_gh(i + 1, h8, l)

                    r_sb = gpool.tile([P, NF], f16, tag="gpool",
                                      name=f"r{i}")
                    nc.scalar.activation(
                        out=r_sb[:], in_=pr[:], func=Sigmoid,
                        scale=DESC, bias=bsum_sb[:, i: i + 1],
                    )
                    z_sb = gpool.tile([P, NF], f16, tag="gpool",
                                      name=f"z{i}")
                    nc.scalar.activation(
                        out=z_sb[:], in_=pz[:], func=Sigmoid,
                        scale=DESC, bias=bsum_sb[:, DCH + i: DCH + i + 1],
                    )
                    rn = gpool.tile([P, NF], f16, tag="gpool")
                    if zero_nb:
                        # rn = (ghn * 2^-12) * r   (zero n-gate biases)
                        nc.vector.scalar_tensor_tensor(
                            out=rn[:], in0=pghn[:], scalar=DESC,
                            in1=r_sb[:], op0=MULT, op1=MULT,
                        )
                    else:
                        hb = gpool.tile([P, NF], f16, tag="gpool")
                        nc.vector.tensor_scalar(
                            out=hb[:], in0=pghn[:], scalar1=DESC,
                            scalar2=bhhn_sb[:, i: i + 1], op0=MULT, op1=ADD,
                        )
                        nc.vector.tensor_mul(out=rn[:], in0=r_sb[:],
                                             in1=hb[:])
                    tn = gpool.tile([P, NF], f16, tag="gpool")
                    if zero_nb:
                        nc.vector.tensor_add(out=tn[:], in0=pgn[:],
                                             in1=rn[:])
                    else:
                        nc.vector.scalar_tensor_tensor(
                            out=tn[:], in0=pgn[:],
                            scalar=bihn_sb[:, i: i + 1],
                            in1=rn[:], op0=ADD, op1=ADD,
                        )
                    nn_ = gpool.tile([P, NF], f16, tag="gpool")
                    nc.scalar.activation(out=nn_[:], in_=tn[:], func=Tanh)
                    # h' = n + z * (h - n)
                    s_ = gpool.tile([P, NF], f16, tag="gpool")
                    nc.vector.tensor_sub(out=s_[:], in0=h16[:, i, :],
                                         in1=nn_[:])
                    sz = gpool.tile([P, NF], f16, tag="gpool")
                    nc.vector.tensor_mul(out=sz[:], in0=z_sb[:], in1=s_[:])
                    nc.vector.tensor_add(out=h16n[:, i, :], in0=nn_[:],
                                         in1=sz[:])
                    if not last:
                        nc.gpsimd.tensor_scalar_mul(
                            out=h8n[:, i, :], in0=h16n[:, i, :], scalar1=S_A,
                        )
                    else:
                        # mask + write out feature-major; host transposes
                        ob = opool.tile([P, NF], f32, tag="opool")
                        nc.vector.tensor_mul(out=ob[:], in0=h16n[:, i, :],
                                             in1=mask_sb[:])
                        nc.sync.dma_start(out=out[i * P: (i + 1) * P, :],
                                          in_=ob[:])
                h16 = h16n
                h8 = h8n

    nc.compile()
    return nc


@functools.lru_cache(maxsize=4)
def _get_nc(pool_wide: bool, zero_nb: bool = True) -> bass.Bass:
    return build_nc(pool_wide, zero_nb)


def _prep_shared(inputs):
    """Weight tensors identical across graphs, pre-laid-out partition-major."""
    import ml_dtypes
    E4 = ml_dtypes.float8_e4m3

    def q8w(x):
        return np.clip(np.asarray(x, np.float32) * S_W, -224.0, 224.0).astype(E4)

    fusion_w = np.ascontiguousarray(np.asarray(inputs["fusion_w"], np.float16))
    fusion_b = np.ascontiguousarray(
        np.asarray(inputs["fusion_b"], np.float32).reshape(DCH, P).T)
    wl = np.ascontiguousarray(
        np.asarray(inputs["ggnn_w"], np.float32)
        .reshape(L, DCH, P, D).transpose(0, 2, 1, 3).astype(np.float16))
    wih_w = np.asarray(inputs["gru_w_ih"], np.float32)   # [K3, D]
    whh_w = np.asarray(inputs["gru_w_hh"], np.float32)
    bih = np.asarray(inputs["gru_b_ih"], np.float32)
    bhh = np.asarray(inputs["gru_b_hh"], np.float32)
    wihT = wih_w.T                                       # [D, K3]
    whhT = whh_w.T
    wihn = np.ascontiguousarray(
        wihT[:, 2 * D:].reshape(DCH, P, D).transpose(1, 0, 2)
        .astype(np.float16))
    wih8 = np.ascontiguousarray(
        q8w(wihT[:, :2 * D]).reshape(DCH, P, 2 * D).transpose(1, 0, 2))
    whh8 = np.ascontiguousarray(
        q8w(whhT).reshape(DCH, P, K3).transpose(1, 0, 2))
    bsum = np.ascontiguousarray((bih + bhh)[:2 * D].reshape(2 * DCH, P).T)
    bihn = np.ascontiguousarray(bih[2 * D:].reshape(DCH, P).T)
    bhhn = np.ascontiguousarray(bhh[2 * D:].reshape(DCH, P).T)
    word_emb = np.ascontiguousarray(np.asarray(inputs["word_emb"], np.float16))
    type_table = np.ascontiguousarray(
        np.asarray(inputs["type_table"], np.float16))
    return dict(
        word_emb=word_emb, type_table=type_table, fusion_w=fusion_w,
        fusion_b=fusion_b, wl=wl, wihn=wihn, wih8=wih8, whh8=whh8,
        bsum=bsum, bihn=bihn, bhhn=bhhn,
    )


def _graph_blockable(inputs, b):
    seg = np.asarray(inputs["token_seg_ids"][b], np.int64)
    tcol = np.arange(T) // P
    return bool(np.all((seg >= tcol * BLK) & (seg < (tcol + 1) * BLK)))


def _prep_graph(inputs, b, pool_wide):
    tok = np.asarray(inputs["node_token_ids"][b], np.int64)
    typ = np.asarray(inputs["node_types"][b], np.int32)
    seg = np.asarray(inputs["token_seg_ids"][b], np.int64)
    lens = np.asarray(inputs["node_token_lens"][b], np.float64)
    glen = int(np.asarray(inputs["graph_node_lens"][b]))
    esrc = np.asarray(inputs["edge_src"][b], np.int64)
    edst = np.asarray(inputs["edge_dst"][b], np.int64)
    ew = np.asarray(inputs["edge_weight"][b], np.float32)

    # token idxs for dma_gather: GS splits of GT idxs, each wrapped into
    # 16 partitions ([p, s] = idx[s*16+p]) and replicated to 128 partitions
    tok16 = tok.astype(np.int16)
    cols = []
    for s in range(GS):
        w16 = tok16[s * GT: (s + 1) * GT].reshape(GT // 16, 16).T
        cols.append(np.tile(w16, (8, 1)))
    tok_idx = np.ascontiguousarray(np.concatenate(cols, axis=1))

    typ_oh = np.zeros((TYPES, N), np.float16)
    typ_oh[typ, np.arange(N)] = 1.0

    # dense transposed adjacency: AT[src, dst], laid out [P, NCH, N]
    at = np.zeros((N, N), np.float32)
    np.add.at(at, (esrc, edst), ew)
    at = np.ascontiguousarray(
        at.reshape(NCH, P, N).transpose(1, 0, 2).astype(np.float16))

    # pooling matrix (1/len weights), [P, TCH, BLK or N]
    winv = np.zeros(N, np.float64)
    nzmask = lens != 0
    winv[nzmask] = 1.0 / lens[nzmask]
    tcol = np.arange(T) // P  # token chunk of each token
    if pool_wide:
        poolm = np.zeros((TCH, P, N), np.float16)
        poolm[tcol, np.arange(T) % P, seg] = winv[seg]
    else:
        poolm = np.zeros((TCH, P, BLK), np.float16)
        poolm[tcol, np.arange(T) % P, seg - tcol * BLK] = winv[seg]
    poolm = np.ascontiguousarray(poolm.transpose(1, 0, 2))

    keep = min(glen, MAX_NODE_LEN)
    mask = np.ascontiguousarray(
        np.tile((np.arange(N) < keep).astype(np.float16), (P, 1)))
    return dict(tok_idx=tok_idx, typ_oh=typ_oh, at_w=at, poolm=poolm,
                maskb=mask)


def kernel(**inputs) -> np.ndarray:
    shared = _prep_shared(inputs)
    pool_wide = not all(_graph_blockable(inputs, b) for b in range(B))
    per_graph = [_prep_graph(inputs, b, pool_wide) for b in range(B)]
    zero_nb = bool(
        np.all(np.asarray(inputs["gru_b_ih"], np.float64)[2 * D:] == 0)
        and np.all(np.asarray(inputs["gru_b_hh"], np.float64)[2 * D:] == 0))
    nc = _get_nc(pool_wide, zero_nb)
    in_maps = [{**shared, **per_graph[b]} for b in range(B)]
    res = bass_utils.run_bass_kernel_spmd(nc, in_maps, core_ids=list(range(B)))
    global _last_exec_ns
    _last_exec_ns = res.exec_time_ns
    # device output is feature-major [D, N]; transpose per graph on host
    out = np.stack([np.asarray(r["out"]).T for r in res.results])
    return np.ascontiguousarray(out.astype(np.float32))


_last_exec_ns = None


# revision 3
# speedup vs baseline: 1.0464x; 1.0031x over previous
# GGNN encoder kernel for Trainium2 (Bass/Tile), data-parallel over the
# batch dimension: 8 graphs -> 8 NeuronCores, one graph per core.
#
# Numerics strategy (validated against the reference in numpy):
#   - all tensors stored fp16 in SBUF, matmuls accumulate in f32 PSUM
#   - the GRU r/z gate matmuls and the gh_n matmul run in fp8e4 (e4m3)
#     with DoubleRow perf mode (2 contraction rows per pass, 2x PE rate);
#     these paths only perturb the output at second order (sigmoid gates
#     multiply small residuals; ghn is damped by r) - measured final
#     rel err ~6.5e-3 vs the 2e-2 gate.
#   - the additive n-path (m, agg, gi_n, fusion, pooling) stays fp16.
#   fp8 scales: activations x16, weights x256, descale 2^-12 folded into
#   the ACT scale of the consuming sigmoid/ident op.
#
# Layout: h, agg feature-major fp16 [128, DCH, 512] with fp8 shadows for
# the DoubleRow matmuls; m node-major fp16 [128, NCH, 768]. All weights
# loaded once (fp8/fp16), never streamed per layer. Output written
# feature-major and transposed on the host.
#
# Engine assignment: PE matmuls; ACT sigmoid/tanh/descale + agg8 cast;
# DVE the GRU elementwise chain (fp16 2x mode); Pool(gpsimd) the
# psum->sbuf evacuations and fp8 casts; SP the weight/output DMAs.

import functools

import numpy as np

import concourse.bass as bass
import concourse.mybir as mybir
import concourse.tile as tile
from concourse import bacc, bass_utils

# Problem shapes (hardcoded: kernel must be self-contained).
B, N, T, D, TD, L = 8, 512, 2048, 768, 128, 4
V, TYPES = 30522, 64
MAX_NODE_LEN = 512
K3 = 3 * D            # 2304 stacked GRU gate rows
F = TD + D            # 896 fused embedding dim
P = 128               # partitions
NCH = N // P          # 4 node chunks
TCH = T // P          # 16 token chunks
DCH = D // P          # 6 feature chunks
FCH = F // P          # 7 fused-dim chunks
BLK = N // TCH        # 32 nodes per token chunk (block-pooling case)
NF = 512              # free-dim tile (nodes)
GS = 4                # token gather splits
GT = T // GS          # tokens per gather split (512)
GC = GT // P          # 128-chunks per gather split (4)

S_A = 16.0            # fp8 activation scale (2^4)
S_W = 256.0           # fp8 weight scale (2^8)
DESC = 1.0 / (S_A * S_W)

f32 = mybir.dt.float32
f16 = mybir.dt.float16
f8 = mybir.dt.float8e4
i16 = mybir.dt.int16

DR = mybir.MatmulPerfMode.DoubleRow
Sigmoid = mybir.ActivationFunctionType.Sigmoid
Tanh = mybir.ActivationFunctionType.Tanh
Ident = mybir.ActivationFunctionType.Identity
Copy = mybir.ActivationFunctionType.Copy
ADD = mybir.AluOpType.add
MULT = mybir.AluOpType.mult


def build_nc(pool_wide: bool, zero_nb: bool = True,
             full_mask: bool = True) -> bass.Bass:
    nc = bacc.Bacc(num_swdge_queues=2, dynamic_dma_scratch_size=32768)

    tok_idx = nc.dram_tensor("tok_idx", [P, GS * (GT // 16)], i16,
                             kind="ExternalInput")
    typ_oh = nc.dram_tensor("typ_oh", [TYPES, N], f16, kind="ExternalInput")
    word_emb = nc.dram_tensor("word_emb", [V, D], f16, kind="ExternalInput")
    type_table = nc.dram_tensor("type_table", [TYPES, TD], f16,
                                kind="ExternalInput")
    pool_w = N if pool_wide else BLK
    poolm = nc.dram_tensor("poolm", [P, TCH, pool_w], f16, kind="ExternalInput")
    at_w = nc.dram_tensor("at_w", [P, NCH, N], f16, kind="ExternalInput")
    fusion_w = nc.dram_tensor("fusion_w", [F, D], f16, kind="ExternalInput")
    fusion_b = nc.dram_tensor("fusion_b", [P, DCH], f32, kind="ExternalInput")
    wl = nc.dram_tensor("wl", [L, P, DCH, D], f16, kind="ExternalInput")
    wihn = nc.dram_tensor("wihn", [P, DCH, D], f16, kind="ExternalInput")
    wih8d = nc.dram_tensor("wih8", [P, DCH, 2 * D], f8, kind="ExternalInput")
    whh8d = nc.dram_tensor("whh8", [P, DCH, K3], f8, kind="ExternalInput")
    bsum = nc.dram_tensor("bsum", [P, 2 * DCH], f32, kind="ExternalInput")
    bihn = nc.dram_tensor("bihn", [P, DCH], f32, kind="ExternalInput")
    bhhn = nc.dram_tensor("bhhn", [P, DCH], f32, kind="ExternalInput")
    maskb = nc.dram_tensor("maskb", [P, N], f16, kind="ExternalInput")
    out = nc.dram_tensor("out", [D, N], f32, kind="ExternalOutput")

    with tile.TileContext(nc) as tc:
        with (
            tc.tile_pool(name="consts", bufs=1) as consts,
            tc.tile_pool(name="wpool", bufs=1) as wpool,
            tc.tile_pool(name="fpool", bufs=1) as fpool,
            tc.tile_pool(name="hpool", bufs=2) as hpool,
            tc.tile_pool(name="apool", bufs=2) as apool,
            tc.tile_pool(name="mpool", bufs=2) as mpool,
            tc.tile_pool(name="gpool", bufs=14) as gpool,
            tc.tile_pool(name="tokg", bufs=4) as tokg,
            tc.tile_pool(name="opool", bufs=2) as opool,
            tc.tile_pool(name="psA", bufs=5, space="PSUM") as psA,
        ):
            # gh psums (tag psG) get their own 3-slot ring so the early-gh
            # emission can pin slots across the agg phase without deadlocking
            # the main psum ring
            def gh_psum(name):
                return psA.tile([P, NF], f32, tag="psG", bufs=3, name=name)
            # ---- token gather first: it gates the whole front of the kernel
            tok_idx_sb = consts.tile([P, T // 16], i16)
            nc.sync.dma_start(out=tok_idx_sb[:], in_=tok_idx[:])
            pool_sb = consts.tile([P, TCH, pool_w], f16)
            nc.sync.dma_start(out=pool_sb[:], in_=poolm[:])

            tt_sb = consts.tile([TYPES, TD], f16)
            nc.sync.dma_start(out=tt_sb[:], in_=type_table[:])
            oh_sb = consts.tile([TYPES, N], f16)
            nc.sync.dma_start(out=oh_sb[:], in_=typ_oh[:])

            fw = []
            for k in range(FCH):
                fwk = fpool.tile([P, D], f16, tag="fw", bufs=FCH,
                                 name=f"fw{k}")
                nc.sync.dma_start(out=fwk[:], in_=fusion_w[k * P: (k + 1) * P, :])
                fw.append(fwk)
            gath = []
            for s in range(GS):
                tg = tokg.tile([P, GC, D], f16, tag="tokg", name=f"tokg{s}")
                nc.gpsimd.dma_gather(
                    tg[:],
                    word_emb[:],
                    tok_idx_sb[:, s * (GT // 16): (s + 1) * (GT // 16)],
                    GT,
                    GT,
                    D,
                    queue_num=s % 2,
                )
                gath.append(tg)

            # ---- remaining constants / weights (after the gathers so the
            # gathers win the DMA engines first) ----
            bsum_sb = consts.tile([P, 2 * DCH], f32)
            nc.sync.dma_start(out=bsum_sb[:], in_=bsum[:])
            bihn_sb = consts.tile([P, DCH], f32)
            nc.sync.dma_start(out=bihn_sb[:], in_=bihn[:])
            bhhn_sb = consts.tile([P, DCH], f32)
            nc.sync.dma_start(out=bhhn_sb[:], in_=bhhn[:])
            fb_sb = consts.tile([P, DCH], f32)
            nc.sync.dma_start(out=fb_sb[:], in_=fusion_b[:])
            mask_sb = consts.tile([P, N], f16)
            nc.sync.dma_start(out=mask_sb[:], in_=maskb[:])

            # weights layer 0 needs first, then the rest
            wl_sb = []
            w0 = wpool.tile([P, DCH, D], f16, tag="wl", bufs=L, name="wl0")
            nc.sync.dma_start(out=w0[:], in_=wl[0])
            wl_sb.append(w0)
            at_sb = wpool.tile([P, NCH, N], f16, tag="at")
            nc.sync.dma_start(out=at_sb[:], in_=at_w[:])
            whh8 = wpool.tile([P, DCH, K3], f8, tag="whh8")
            nc.sync.dma_start(out=whh8[:], in_=whh8d[:])
            wih8 = wpool.tile([P, DCH, 2 * D], f8, tag="wih8")
            nc.sync.dma_start(out=wih8[:], in_=wih8d[:])
            wihn_sb = wpool.tile([P, DCH, D], f16, tag="wihn")
            nc.sync.dma_start(out=wihn_sb[:], in_=wihn[:])
            for l in range(1, L):
                w = wpool.tile([P, DCH, D], f16, tag="wl", bufs=L,
                               name=f"wl{l}")
                nc.sync.dma_start(out=w[:], in_=wl[l])
                wl_sb.append(w)

            # ---- pooling: per-feature psums accumulate all token chunks;
            # one evac per feature chunk. fusedT[k] is [feat 128, nodes 512].
            fusedT = [fpool.tile([P, NF], f16, tag="fu", bufs=FCH,
                                 name=f"fusedT{k}") for k in range(FCH)]

            ptyp = psA.tile([P, NF], f32, tag="psA")
            nc.tensor.matmul(out=ptyp[:], lhsT=tt_sb[:], rhs=oh_sb[:],
                             start=True, stop=True)
            nc.vector.tensor_copy(out=fusedT[0][:], in_=ptyp[:])

            h16 = hpool.tile([P, DCH, NF], f16, tag="h16", name="h_init")
            h8 = hpool.tile([P, DCH, NF], f8, tag="h8", name="h8_init")
            if pool_wide:
                for f in range(DCH):
                    pcf = psA.tile([P, NF], f32, tag="psG", bufs=3,
                                   name=f"pool{f}")
                    for s in range(GS):
                        for c2 in range(GC):
                            c = s * GC + c2
                            nc.tensor.matmul(
                                out=pcf[:],
                                lhsT=gath[s][:, c2, f * P: (f + 1) * P],
                                rhs=pool_sb[:, c, :],
                                start=(c == 0),
                                stop=(c == TCH - 1),
                            )
                    nc.vector.tensor_copy(out=fusedT[1 + f][:], in_=pcf[:])
                for j in range(DCH):
                    pf = psA.tile([P, NF], f32, tag="psA")
                    for k in range(FCH):
                        nc.tensor.matmul(
                            out=pf[:],
                            lhsT=fw[k][:, j * P: (j + 1) * P],
                            rhs=fusedT[k][:],
                            start=(k == 0),
                            stop=(k == FCH - 1),
                        )
                    nc.vector.tensor_scalar(
                        out=h16[:, j, :], in0=pf[:],
                        scalar1=fb_sb[:, j: j + 1], scalar2=None, op0=ADD,
                    )
                    nc.gpsimd.tensor_scalar_mul(
                        out=h8[:, j, :], in0=h16[:, j, :], scalar1=S_A,
                    )
            else:
                # block pooling: each gather split covers a 128-node block,
                # so pooling, fusion and the h evacuation pipeline per split
                # while later gathers are still in flight
                SB = GC * BLK  # 128 nodes per split
                for s in range(GS):
                    for f in range(DCH):
                        pst = psA.tile([P, SB], f32, tag="psG", bufs=3,
                                       name=f"pool{f}_{s}")
                        for c2 in range(GC):
                            nc.tensor.matmul(
                                out=pst[:, c2 * BLK: (c2 + 1) * BLK],
                                lhsT=gath[s][:, c2, f * P: (f + 1) * P],
                                rhs=pool_sb[:, s * GC + c2, :],
                                start=True,
                                stop=True,
                            )
                        nc.vector.tensor_copy(
                            out=fusedT[1 + f][:, s * SB: (s + 1) * SB],
                            in_=pst[:])
                    for j in range(DCH):
                        pf = psA.tile([P, SB], f32, tag="psA",
                                      name=f"pf{j}_{s}")
                        for k in range(FCH):
                            nc.tensor.matmul(
                                out=pf[:],
                                lhsT=fw[k][:, j * P: (j + 1) * P],
                                rhs=fusedT[k][:, s * SB: (s + 1) * SB],
                                start=(k == 0),
                                stop=(k == FCH - 1),
                            )
                        nc.vector.tensor_scalar(
                            out=h16[:, j, s * SB: (s + 1) * SB], in0=pf[:],
                            scalar1=fb_sb[:, j: j + 1], scalar2=None,
                            op0=ADD,
                        )
                for j in range(DCH):
                    nc.gpsimd.tensor_scalar_mul(
                        out=h8[:, j, :], in0=h16[:, j, :], scalar1=S_A,
                    )

            # ---- GGNN layers ----
            def emit_gh(i, h8t, lbl):
                """fp8 DoubleRow gh matmuls for gate chunk i; r/z psums stay
                open (gi accumulates into them later)."""
                przn = []
                for g in range(3):
                    pg = gh_psum(f"gh{lbl}_{g}_{i}")
                    col = g * D + i * P
                    for k in range(0, DCH, 2):
                        nc.tensor.matmul(
                            out=pg[:],
                            lhsT=whh8[:, k: k + 2, col: col + P],
                            rhs=h8t[:, k: k + 2, :],
                            start=(k == 0),
                            stop=(g == 2 and k + 2 == DCH),
                            perf_mode=DR,
                        )
                    przn.append(pg)
                return przn

            def emit_m(h16t, wlk, lbl, kouter):
                # m = h @ Wl  (node-major out [node 128, 768] per chunk).
                # k-outer form starts contracting as soon as the first h
                # chunks are written, filling the PE during the previous
                # layer's elementwise tail.
                m16 = mpool.tile([P, NCH, D], f16, tag="m16",
                                 name=f"m{lbl}")
                halves = [(0, 1), (2, 3)] if kouter else [(i,) for i in
                                                          range(NCH)]
                for half in halves:
                    pms = []
                    for i in half:
                        pma = psA.tile([P, NF], f32, tag="psA",
                                       name=f"pma{lbl}_{i}")
                        pmb = psA.tile([P, D - NF], f32, tag="psA",
                                       name=f"pmb{lbl}_{i}")
                        pms.append((pma, pmb))
                    for k in range(DCH):
                        for x, i in enumerate(half):
                            pma, pmb = pms[x]
                            nc.tensor.matmul(
                                out=pma[:],
                                lhsT=h16t[:, k, i * P: (i + 1) * P],
                                rhs=wlk[:, k, :NF],
                                start=(k == 0),
                                stop=(k == DCH - 1),
                            )
                            nc.tensor.matmul(
                                out=pmb[:],
                                lhsT=h16t[:, k, i * P: (i + 1) * P],
                                rhs=wlk[:, k, NF:D],
                                start=(k == 0),
                                stop=(k == DCH - 1),
                            )
                    for x, i in enumerate(half):
                        pma, pmb = pms[x]
                        nc.scalar.activation(out=m16[:, i, :NF], in_=pma[:],
                                             func=Copy)
                        nc.scalar.activation(out=m16[:, i, NF:D], in_=pmb[:],
                                             func=Copy)
                return m16

            m16 = emit_m(h16, wl_sb[0], 0, False)
            for l in range(L):
                last = l == L - 1

                # gh for chunk 0: needs only h8, keeps the PE busy while the
                # m evacuations drain before agg can start
                gh_next = emit_gh(0, h8, l)

                # aggT = m.T @ A.T  (feature-major [feat 128, nodes 512])
                agg16 = apool.tile([P, DCH, NF], f16, tag="a16",
                                   name=f"agg{l}")
                agg8 = apool.tile([P, DCH, NF], f8, tag="a8",
                                  name=f"agg8_{l}")
                for j in range(DCH):
                    pa = psA.tile([P, NF], f32, tag="psA")
                    for k in range(NCH):
                        nc.tensor.matmul(
                            out=pa[:],
                            lhsT=m16[:, k, j * P: (j + 1) * P],
                            rhs=at_sb[:, k, :],
                            start=(k == 0),
                            stop=(k == NCH - 1),
                        )
                    nc.scalar.activation(out=agg8[:, j, :], in_=pa[:],
                                         func=Copy, scale=S_A)
                    nc.vector.tensor_copy(out=agg16[:, j, :], in_=pa[:])

                # GRU gates, 128 gate rows at a time
                h16n = hpool.tile([P, DCH, NF], f16, tag="h16",
                                  name=f"h{l + 1}")
                h8n = hpool.tile([P, DCH, NF], f8, tag="h8",
                                 name=f"h8_{l + 1}")
                for i in range(DCH):
                    pr, pz, pghn = gh_next
                    # gi_r, gi_z: fp8 DoubleRow accumulating into gh psums
                    for g, pg in ((0, pr), (1, pz)):
                        col = g * D + i * P
                        for k in range(0, DCH, 2):
                            nc.tensor.matmul(
                                out=pg[:],
                                lhsT=wih8[:, k: k + 2, col: col + P],
                                rhs=agg8[:, k: k + 2, :],
                                start=False,
                                stop=(k + 2 == DCH),
                                perf_mode=DR,
                            )
                    # gi_n: fp16 (precision-critical additive path)
                    pgn = psA.tile([P, NF], f32, tag="psA", name=f"pgin{i}")
                    for k in range(DCH):
                        nc.tensor.matmul(
                            out=pgn[:],
                            lhsT=wihn_sb[:, k, i * P: (i + 1) * P],
                            rhs=agg16[:, k, :],
                            start=(k == 0),
                            stop=(k == DCH - 1),
                        )
                    if i + 1 < DCH:
                        gh_next = emit_gh(i + 1, h8, l)

                    # evacuate/descale ghn on ACT right away so its psum
                    # slot frees before the next chunk's gh matmuls need it
                    hb = gpool.tile([P, NF], f16, tag="gpool")
                    if zero_nb:
                        nc.scalar.activation(out=hb[:], in_=pghn[:],
                                             func=Copy, scale=DESC)
                    else:
                        nc.scalar.activation(
                            out=hb[:], in_=pghn[:], func=Ident, scale=DESC,
                            bias=bhhn_sb[:, i: i + 1],
                        )
                    r_sb = gpool.tile([P, NF], f16, tag="gpool",
                                      name=f"r{i}")
                    nc.scalar.activation(
                        out=r_sb[:], in_=pr[:], func=Sigmoid,
                        scale=DESC, bias=bsum_sb[:, i: i + 1],
                    )
                    z_sb = gpool.tile([P, NF], f16, tag="gpool",
                                      name=f"z{i}")
                    nc.scalar.activation(
                        out=z_sb[:], in_=pz[:], func=Sigmoid,
                        scale=DESC, bias=bsum_sb[:, DCH + i: DCH + i + 1],
                    )
                    rn = gpool.tile([P, NF], f16, tag="gpool")
                    nc.vector.tensor_mul(out=rn[:], in0=r_sb[:], in1=hb[:])
                    tn = gpool.tile([P, NF], f16, tag="gpool")
                    if zero_nb:
                        nc.vector.tensor_add(out=tn[:], in0=pgn[:],
                                             in1=rn[:])
                    else:
                        nc.vector.scalar_tensor_tensor(
                            out=tn[:], in0=pgn[:],
                            scalar=bihn_sb[:, i: i + 1],
                            in1=rn[:], op0=ADD, op1=ADD,
                        )
                    nn_ = gpool.tile([P, NF], f16, tag="gpool")
                    nc.scalar.activation(out=nn_[:], in_=tn[:], func=Tanh)
                    # h' = n + z * (h - n)
                    s_ = gpool.tile([P, NF], f16, tag="gpool")
                    nc.vector.tensor_sub(out=s_[:], in0=h16[:, i, :],
                                         in1=nn_[:])
                    sz = gpool.tile([P, NF], f16, tag="gpool")
                    nc.vector.tensor_mul(out=sz[:], in0=z_sb[:], in1=s_[:])
                    if last and full_mask:
                        # fuse h' = n + z*(h-n) with the f32 output write;
                        # all nodes valid so the mask multiply is skipped
                        ob = opool.tile([P, NF], f32, tag="opool")
                        nc.vector.tensor_add(out=ob[:], in0=nn_[:],
                                             in1=sz[:])
                        nc.sync.dma_start(out=out[i * P: (i + 1) * P, :],
                                          in_=ob[:])
                        continue
                    nc.vector.tensor_add(out=h16n[:, i, :], in0=nn_[:],
                                         in1=sz[:])
                    if not last:
                        nc.gpsimd.tensor_scalar_mul(
                            out=h8n[:, i, :], in0=h16n[:, i, :], scalar1=S_A,
                        )
                    else:
                        # mask + write out feature-major; host transposes
                        ob = opool.tile([P, NF], f32, tag="opool")
                        nc.vector.tensor_mul(out=ob[:], in0=h16n[:, i, :],
                                             in1=mask_sb[:])
                        nc.sync.dma_start(out=out[i * P: (i + 1) * P, :],
                                          in_=ob[:])
                if not last:
                    m16 = emit_m(h16n, wl_sb[l + 1], l + 1, True)
                h16 = h16n
                h8 = h8n

    nc.compile()
    return nc


@functools.lru_cache(maxsize=4)
def _get_nc(pool_wide: bool, zero_nb: bool = True,
            full_mask: bool = True) -> bass.Bass:
    return build_nc(pool_wide, zero_nb, full_mask)


def _prep_shared(inputs):
    """Weight tensors identical across graphs, pre-laid-out partition-major."""
    import ml_dtypes
    E4 = ml_dtypes.float8_e4m3

    def q8w(x):
        return np.clip(np.asarray(x, np.float32) * S_W, -224.0, 224.0).astype(E4)

    fusion_w = np.ascontiguousarray(np.asarray(inputs["fusion_w"], np.float16))
    fusion_b = np.ascontiguousarray(
        np.asarray(inputs["fusion_b"], np.float32).reshape(DCH, P).T)
    wl = np.ascontiguousarray(
        np.asarray(inputs["ggnn_w"], np.float32)
        .reshape(L, DCH, P, D).transpose(0, 2, 1, 3).astype(np.float16))
    wih_w = np.asarray(inputs["gru_w_ih"], np.float32)   # [K3, D]
    whh_w = np.asarray(inputs["gru_w_hh"], np.float32)
    bih = np.asarray(inputs["gru_b_ih"], np.float32)
    bhh = np.asarray(inputs["gru_b_hh"], np.float32)
    wihT = wih_w.T                                       # [D, K3]
    whhT = whh_w.T
    wihn = np.ascontiguousarray(
        wihT[:, 2 * D:].reshape(DCH, P, D).transpose(1, 0, 2)
        .astype(np.float16))
    wih8 = np.ascontiguousarray(
        q8w(wihT[:, :2 * D]).reshape(DCH, P, 2 * D).transpose(1, 0, 2))
    whh8 = np.ascontiguousarray(
        q8w(whhT).reshape(DCH, P, K3).transpose(1, 0, 2))
    bsum = np.ascontiguousarray((bih + bhh)[:2 * D].reshape(2 * DCH, P).T)
    bihn = np.ascontiguousarray(bih[2 * D:].reshape(DCH, P).T)
    bhhn = np.ascontiguousarray(bhh[2 * D:].reshape(DCH, P).T)
    word_emb = np.ascontiguousarray(np.asarray(inputs["word_emb"], np.float16))
    type_table = np.ascontiguousarray(
        np.asarray(inputs["type_table"], np.float16))
    return dict(
        word_emb=word_emb, type_table=type_table, fusion_w=fusion_w,
        fusion_b=fusion_b, wl=wl, wihn=wihn, wih8=wih8, whh8=whh8,
        bsum=bsum, bihn=bihn, bhhn=bhhn,
    )


def _graph_blockable(inputs, b):
    seg = np.asarray(inputs["token_seg_ids"][b], np.int64)
    tcol = np.arange(T) // P
    return bool(np.all((seg >= tcol * BLK) & (seg < (tcol + 1) * BLK)))


def _prep_graph(inputs, b, pool_wide):
    tok = np.asarray(inputs["node_token_ids"][b], np.int64)
    typ = np.asarray(inputs["node_types"][b], np.int32)
    seg = np.asarray(inputs["token_seg_ids"][b], np.int64)
    lens = np.asarray(inputs["node_token_lens"][b], np.float64)
    glen = int(np.asarray(inputs["graph_node_lens"][b]))
    esrc = np.asarray(inputs["edge_src"][b], np.int64)
    edst = np.asarray(inputs["edge_dst"][b], np.int64)
    ew = np.asarray(inputs["edge_weight"][b], np.float32)

    # token idxs for dma_gather: GS splits of GT idxs, each wrapped into
    # 16 partitions ([p, s] = idx[s*16+p]) and replicated to 128 partitions
    tok16 = tok.astype(np.int16)
    cols = []
    for s in range(GS):
        w16 = tok16[s * GT: (s + 1) * GT].reshape(GT // 16, 16).T
        cols.append(np.tile(w16, (8, 1)))
    tok_idx = np.ascontiguousarray(np.concatenate(cols, axis=1))

    typ_oh = np.zeros((TYPES, N), np.float16)
    typ_oh[typ, np.arange(N)] = 1.0

    # dense transposed adjacency: AT[src, dst], laid out [P, NCH, N]
    at = np.zeros((N, N), np.float32)
    np.add.at(at, (esrc, edst), ew)
    at = np.ascontiguousarray(
        at.reshape(NCH, P, N).transpose(1, 0, 2).astype(np.float16))

    # pooling matrix (1/len weights), [P, TCH, BLK or N]
    winv = np.zeros(N, np.float64)
    nzmask = lens != 0
    winv[nzmask] = 1.0 / lens[nzmask]
    tcol = np.arange(T) // P  # token chunk of each token
    if pool_wide:
        poolm = np.zeros((TCH, P, N), np.float16)
        poolm[tcol, np.arange(T) % P, seg] = winv[seg]
    else:
        poolm = np.zeros((TCH, P, BLK), np.float16)
        poolm[tcol, np.arange(T) % P, seg - tcol * BLK] = winv[seg]
    poolm = np.ascontiguousarray(poolm.transpose(1, 0, 2))

    keep = min(glen, MAX_NODE_LEN)
    mask = np.ascontiguousarray(
        np.tile((np.arange(N) < keep).astype(np.float16), (P, 1)))
    return dict(tok_idx=tok_idx, typ_oh=typ_oh, at_w=at, poolm=poolm,
                maskb=mask)


def kernel(**inputs) -> np.ndarray:
    shared = _prep_shared(inputs)
    pool_wide = not all(_graph_blockable(inputs, b) for b in range(B))
    per_graph = [_prep_graph(inputs, b, pool_wide) for b in range(B)]
    zero_nb = bool(
        np.all(np.asarray(inputs["gru_b_ih"], np.float64)[2 * D:] == 0)
        and np.all(np.asarray(inputs["gru_b_hh"], np.float64)[2 * D:] == 0))
    full_mask = bool(
        np.all(np.asarray(inputs["graph_node_lens"], np.int64)
               >= min(N, MAX_NODE_LEN)))
    nc = _get_nc(pool_wide, zero_nb, full_mask)
    in_maps = [{**shared, **per_graph[b]} for b in range(B)]
    res = bass_utils.run_bass_kernel_spmd(nc, in_maps, core_ids=list(range(B)))
    global _last_exec_ns
    _last_exec_ns = res.exec_time_ns
    # device output is feature-major [D, N]; transpose per graph on host
    out = np.stack([np.asarray(r["out"]).T for r in res.results])
    return np.ascontiguousarray(out.astype(np.float32))


_last_exec_ns = None
